# revision 27
# baseline (speedup 1.0000x reference)
"""BIDAF attention-flow kernel for Trainium2 (Bass/Tile), 8-core data-parallel.

Reference computation (per batch b):
    S[t,j]  = H[t]·w_h + U[j]·w_u + sum_d H[t,d]*U[j,d]*w_hu[d]
    A       = softmax_j(S);          C2Q = A @ U
    b_att   = softmax_t(max_j S);    Q2C = b_att @ H   (broadcast over t)
    G       = [H, C2Q, H*C2Q, H*Q2C]        # [T, 4D]

Kernel strategy (per core, 8 batches):
  * S is computed TRANSPOSED (ST[j,t]) so the C2Q matmul consumes P=exp(ST)
    directly as lhsT, and the moving dim is T (N=512 -> fp32r full rate).
  * sh[t]=H·w_h is folded into the similarity matmul as a rank-1 update:
    Uw' = U*w_hu + w_h (broadcast over j).  sh is constant over j, so
    softmax_j is unchanged, and max_j exp(S') == b_att weights wq directly —
    the separate sh/esh/wq pipeline disappears.
  * su[j]=U·w_u enters as the per-partition ACT bias of the exp.
  * Ones-columns appended host-side (col 256 of the 260-wide inputs) give
    l[t]=sum_j P and Wsum=sum_t wq free inside the C2Q/Q2C matmuls; the
    normalizers fold into the mandatory PSUM->SBUF copies.
  * max_j P needs a partition reduce: PE re-transposes P in [128,128] blocks
    and DVE reduce_max's each half.
  * H is loaded once as fp32r (engines read it through f32 bitcast views) —
    no SBUF->SBUF duplicate.  w_hu/w_h are applied per-partition AFTER the
    U transpose (d on partitions) with a single fused DVE tensor_scalar.
  * Input loads issue on the scalar HWDGE ring, output stores on the sync
    ring, so next-batch loads never queue behind this batch's stores.
  * PSUM: st(1) + htp(2) + pt(2) + cq(2) + small(1) = 8 banks, with
    ping-pong on the pipelined tags.
  * Tile emits multi-wait instructions; TRN2 allows 1 wait/instruction, so
    the bacc rust passes are run on the traced module before compile.
"""

import sys

sys.path.insert(0, "/opt/trn_rl_repo")

import numpy as np

import concourse.bass as bass
import concourse.mybir as mybir
from concourse import tile

B, T, J, D = 64, 1024, 128, 256
NCORES = 8
BPC = B // NCORES  # batches per core
P = 128
NT = T // P  # 8 t-chunks per batch
DA = 260  # augmented feature dim: [x | 1 | pad(1.0)*3]
F32 = mybir.dt.float32
F32R = mybir.dt.float32r
AF = mybir.ActivationFunctionType
ALU = mybir.AluOpType
AX = mybir.AxisListType

# float32r streams fp32 bits through the PE at 1 cycle/row for N>=256.
MMDT = F32R


def build_kernel(nc, bpc):
    H = nc.declare_dram_parameter("H", [bpc, T, DA], F32, isOutput=False)
    U = nc.declare_dram_parameter("U", [bpc, J, DA], F32, isOutput=False)
    # packed constants [128, 133]: cols 0-127 ident, col 128 all-ones,
    # cols 129-130 w_hu (by kc), 131-132 w_h — one DMA; host packs in run().
    const_in = nc.declare_dram_parameter("constp", [P, 133], F32, isOutput=False)
    wu_in = nc.declare_dram_parameter("wu", [1, D], F32, isOutput=False)
    G = nc.declare_dram_parameter("G", [bpc, T, 4 * D], F32, isOutput=True)

    with tile.TileContext(nc) as tc:
        with (
            tc.tile_pool(name="const", bufs=1) as const_pool,
            tc.tile_pool(name="h", bufs=2) as h_pool,
            tc.tile_pool(name="ht", bufs=2) as ht_pool,
            tc.tile_pool(name="p", bufs=2) as p_pool,
            tc.tile_pool(name="g", bufs=2) as g_pool,
            tc.tile_pool(name="u", bufs=2) as u_pool,
            tc.tile_pool(name="sm", bufs=2) as sm_pool,
            tc.tile_pool(name="stps", bufs=1, space="PSUM") as st_ps,
            tc.tile_pool(name="htps", bufs=2, space="PSUM") as htp_ps,
            tc.tile_pool(name="ptps", bufs=2, space="PSUM") as pt_ps,
            tc.tile_pool(name="cqps", bufs=2, space="PSUM") as cq_ps,
            tc.tile_pool(name="smps", bufs=1, space="PSUM") as sm_ps,
        ):
            # ---- constants: one packed DMA on sync, wu row on scalar ----
            cpk = const_pool.tile([P, 133], MMDT)
            nc.sync.dma_start(cpk[:], const_in[:].bitcast(MMDT))
            wu_row = const_pool.tile([1, D], MMDT)
            nc.scalar.dma_start(wu_row[:], wu_in[:].bitcast(MMDT))
            ident = cpk[:, 0:P]
            ones1 = cpk[0:1, 128:129].broadcast_to((1, P))
            wc = cpk[:, 129:133].bitcast(F32).rearrange("p (w kc) -> p kc w", kc=2)
            # broadcast w_u across partitions via a K=1 ones-matmul
            wu_ps = sm_ps.tile([P, D], F32, tag="sm")
            nc.tensor.matmul(wu_ps[:], ones1, wu_row[:], start=True, stop=True)
            wu_b = const_pool.tile([P, D], F32)
            nc.scalar.copy(wu_b[:], wu_ps[:])

            hn_tiles = {}
            upre_tiles = {}

            def issue_loads(b, eng=None):
                eng = eng or nc.scalar
                Hn = h_pool.tile([P, NT, DA], MMDT, name="Hn", bufs=4)
                eng.dma_start(
                    Hn[:], H[b].rearrange("(c p) d -> p c d", p=P).bitcast(MMDT)
                )
                Uo = u_pool.tile([P, DA], MMDT, name="Uo", tag="uo", bufs=4)
                eng.dma_start(Uo[:], U[b].bitcast(MMDT))
                hn_tiles[b] = (Hn, Uo)

            def upre(b):
                # U-side prep: UwT[d,j] = U^T * w_hu[d] + w_h[d], su = U.w_u.
                # Emitted one batch AHEAD so the PE rolls from batch b-1's
                # Q2C matmuls straight into batch b's transposes.
                Uo = hn_tiles[b][1]
                ut = htp_ps.tile([P, 2, P], MMDT, tag="htp", name="ut")
                for kc in range(2):
                    nc.tensor.transpose(
                        ut[:, kc, :], Uo[:, kc * P : (kc + 1) * P], ident
                    )
                UwT = u_pool.tile([P, 2, P], MMDT, tag="uwt")
                for kc in range(2):
                    nc.scalar.activation(
                        UwT[:, kc, :],
                        ut[:, kc, :].bitcast(F32),
                        AF.Identity,
                        bias=wc[:, kc, 1:2],
                        scale=wc[:, kc, 0:1],
                    )
                su = sm_pool.tile([P, 1], F32, tag="su")
                scr = u_pool.tile([P, D], F32, tag="scr")
                nc.gpsimd.tensor_mul(scr[:], Uo[:, 0:D].bitcast(F32), wu_b[:])
                nc.vector.reduce_sum(su[:], scr[:], axis=AX.X)
                upre_tiles[b] = (UwT, su)

            for pb in range(min(3, bpc)):
                issue_loads(pb, eng=(nc.sync if pb == 1 else nc.scalar))
            upre(0)
            for b in range(bpc):
                if b + 3 < bpc:
                    issue_loads(b + 3)
                Hn, Uo = hn_tiles[b]
                Gb = G[b].rearrange("(c p) (g d) -> p c g d", p=P, d=D)
                if b == 0:
                    # G block 0 = H (write out as soon as it is on chip)
                    nc.scalar.dma_start(Gb[:, :, 0, :], Hn[:, :, 0:D].bitcast(F32))
                UwT, su = upre_tiles.pop(b)

                # ---- H transpose + similarity matmul + exp, in T-halves ----
                HT = ht_pool.tile([P, 2, T], MMDT)
                Pt = p_pool.tile([P, T], MMDT)
                st = {}
                for th in range(2):
                    for kc in range(2):
                        htp = htp_ps.tile([P, 512], MMDT, tag="htp")
                        for i in range(4):
                            c = th * 4 + i
                            nc.tensor.transpose(
                                htp[:, i * P : (i + 1) * P],
                                Hn[:, c, kc * P : (kc + 1) * P],
                                ident,
                            )
                        dst = HT[:, kc, th * 512 : (th + 1) * 512]
                        if kc == 0:
                            nc.scalar.copy(dst, htp[:])
                        else:
                            nc.vector.tensor_copy(dst, htp[:])
                    st[th] = st_ps.tile([P, 512], F32, tag="st", name="st")
                    for kc in range(2):
                        nc.tensor.matmul(
                            st[th][:],
                            UwT[:, kc, :],
                            HT[:, kc, th * 512 : (th + 1) * 512],
                            start=(kc == 0),
                            stop=(kc == 1),
                        )
                    # P = exp(shu + sh[t] + su[j]); sh rides in via UwT
                    nc.scalar.activation(
                        Pt[:, th * 512 : (th + 1) * 512],
                        st[th][:],
                        AF.Exp,
                        bias=su[:],
                        scale=1.0,
                    )

                # ---- max_j P (-> wq) via PE transpose + DVE reduce,
                # and C2Q = softmax_j(S)-matmul, pipelined per half ----
                mx = sm_pool.tile([P, NT], F32, tag="mx")
                mxr = sm_pool.tile([P, NT], MMDT, tag="mxr")
                linv = sm_pool.tile([P, NT], F32, tag="linv")
                Gt = g_pool.tile([P, NT, 3, D], F32)
                q2cu = sm_ps.tile([1, 258], F32, tag="sm")
                for th in range(2):
                    pt = pt_ps.tile([P, 512], MMDT, tag="pt")
                    for i in range(4):
                        c = th * 4 + i
                        nc.tensor.transpose(
                            pt[:, i * P : (i + 1) * P],
                            Pt[:, c * P : (c + 1) * P],
                            ident,
                        )
                    nc.vector.reduce_max(
                        mx[:, th * 4 : (th + 1) * 4].unsqueeze(2),
                        pt[:].bitcast(F32).rearrange("p (c j) -> p c j", j=P),
                        axis=AX.X,
                    )
                    nc.vector.tensor_copy(
                        mxr[:, th * 4 : (th + 1) * 4],
                        mx[:, th * 4 : (th + 1) * 4],
                    )
                    for c in range(th * 4, th * 4 + 4):
                        nc.tensor.matmul(
                            q2cu[:],
                            mxr[:, c : c + 1],
                            Hn[:, c, 0:258],
                            start=(c == 0),
                            stop=(c == NT - 1),
                        )
                    for i in range(4):
                        c = th * 4 + i
                        cq = cq_ps.tile([P, 258], F32, tag="cq")
                        nc.tensor.matmul(
                            cq[:],
                            Pt[:, c * P : (c + 1) * P],
                            Uo[:, 0:258],
                            start=True,
                            stop=True,
                        )
                        nc.vector.reciprocal(linv[:, c : c + 1], cq[:, 256:257])
                        if c % 2 == 0:
                            nc.scalar.activation(
                                Gt[:, c, 0, :],
                                cq[:, 0:D],
                                AF.Copy,
                                scale=linv[:, c : c + 1],
                            )
                        else:
                            nc.vector.tensor_scalar_mul(
                                Gt[:, c, 0, :], cq[:, 0:D], linv[:, c : c + 1]
                            )
                    # drain C2Q half; G2 = H * C2Q per half on alternate engines
                    lo, hi = th * 4, th * 4 + 4
                    nc.scalar.dma_start(Gb[:, lo:hi, 1, :], Gt[:, lo:hi, 0, :])
                    eng = nc.gpsimd if th == 0 else nc.vector
                    eng.tensor_mul(
                        Gt[:, lo:hi, 1, :],
                        Hn[:, lo:hi, 0:D].bitcast(F32),
                        Gt[:, lo:hi, 0, :],
                    )
                    nc.sync.dma_start(Gb[:, lo:hi, 2, :], Gt[:, lo:hi, 1, :])

                # ---- Q2C tail: normalize and broadcast ----
                rin = sm_pool.tile([1, 1], F32, tag="rin")
                nc.vector.reciprocal(rin[:], q2cu[:, 256:257])
                q2cn = sm_pool.tile([1, D], MMDT, tag="q2cn")
                nc.scalar.activation(
                    q2cn[:], q2cu[:, 0:D], AF.Copy, scale=rin[:]
                )
                # broadcast Q2C across partitions with a K=1 ones-matmul
                qb_ps = sm_ps.tile([P, D], F32, tag="sm")
                nc.tensor.matmul(qb_ps[:], ones1, q2cn[:], start=True, stop=True)
                q2cb = sm_pool.tile([P, D], F32, tag="q2cb")
                nc.scalar.copy(q2cb[:], qb_ps[:])

                # U-prep for the NEXT batch goes out before this batch's
                # G3 tail so the PE and DVE never drain.
                if b + 1 < bpc:
                    upre(b + 1)
                hn_tiles.pop(b)

                # ---- G3 = H * Q2C (free-dim broadcast of q2cb) ----
                nc.vector.tensor_mul(
                    Gt[:, 0:4, 2, :],
                    Hn[:, 0:4, 0:D].bitcast(F32),
                    q2cb[:].unsqueeze(1).broadcast_to((P, 4, D)),
                )
                nc.sync.dma_start(Gb[:, 0:4, 3, :], Gt[:, 0:4, 2, :])
                nc.gpsimd.tensor_mul(
                    Gt[:, 4:8, 2, :],
                    Hn[:, 4:8, 0:D].bitcast(F32),
                    q2cb[:].unsqueeze(1).broadcast_to((P, 4, D)),
                )
                nc.sync.dma_start(Gb[:, 4:8, 3, :], Gt[:, 4:8, 2, :])
                if b + 1 < bpc:
                    # next batch's H block drains now: its load landed earlier
                    Gb1 = G[b + 1].rearrange("(c p) (g d) -> p c g d", p=P, d=D)
                    nc.scalar.dma_start(
                        Gb1[:, :, 0, :], hn_tiles[b + 1][0][:, :, 0:D].bitcast(F32)
                    )

    return nc


_NC_CACHE = {}


def get_nc(bpc=BPC):
    key = bpc
    if key not in _NC_CACHE:
        import bass_rust as _bass_rust

        nc = bass.Bass()
        build_kernel(nc, bpc)
        # TRN2 allows at most 1 sync wait per instruction (2 on event
        # semaphores); Tile emits more.  These are the bacc lowering passes
        # that legalize the wait lists.
        _bass_rust.move_matmul_waits_to_ldweights(nc.m)
        _bass_rust.generate_event_semaphores(nc)
        # lower bass_isa subclasses (e.g. EVENT_SEMAPHORE_RANGE_CLEAR) into
        # raw InstISA encodings walrus can emit
        mybir.codegen_inst_isa_subclasses(nc)
        _NC_CACHE[key] = nc
    return _NC_CACHE[key]


def _augment(x):
    """[..., D] f32 -> [..., DA] with column D = 1.0 (rest pad 1.0)."""
    out = np.ones(x.shape[:-1] + (DA,), dtype=np.float32)
    out[..., :D] = x
    return out


def run(inputs, trace=False, **kwargs):
    from concourse.bass_utils import run_bass_kernel_spmd

    nc = get_nc(BPC)
    H = _augment(np.asarray(inputs["H"], dtype=np.float32))
    U = _augment(np.asarray(inputs["U"], dtype=np.float32))
    w_h = np.asarray(inputs["w_h"], dtype=np.float32)
    w_hu = np.asarray(inputs["w_hu"], dtype=np.float32)
    wu = np.asarray(inputs["w_u"], dtype=np.float32).reshape(1, D)
    constp = np.ones((P, 133), dtype=np.float32)
    constp[:, 0:P] = np.eye(P, dtype=np.float32)
    constp[:, 129:131] = w_hu.reshape(2, P).T
    constp[:, 131:133] = w_h.reshape(2, P).T
    in_maps = [
        {
            "H": H[c * BPC : (c + 1) * BPC],
            "U": U[c * BPC : (c + 1) * BPC],
            "constp": constp,
            "wu": wu,
        }
        for c in range(NCORES)
    ]
    res = run_bass_kernel_spmd(
        nc, in_maps, core_ids=list(range(NCORES)), trace=trace, **kwargs
    )
    out = np.concatenate([res.results[c]["G"] for c in range(NCORES)], axis=0)
    return out, res


def kernel(**inputs):
    out, _ = run(inputs, trace=False)
    return out
